# revision 1
# baseline (speedup 1.0000x reference)
"""Trainium2 Bass kernel for the DFBL (Gabor filterbank + Kaiser pooling + PCEN) model.

Contract: kernel(**inputs) takes the FULL unsharded inputs
(x [8,1,160000], six [64] param vectors) and returns the FULL output
[8, 64, 1000] float32. Internally shards batch across 8 NeuronCores.

Algorithm (per core, one batch element):
  1. Gabor conv as matmuls via the residue decomposition t = 128u + s:
     out[n, 128u+s] = sum_d Wsd[q,n].T @ x2[q, u+d], where
     x2[q, c] = xpad[128c + q] is a time-minor layout of x loaded once
     (no im2col DMA blowup), and Wsd are host-built 128x128 bf16 weight
     tiles (real|imag channel pairs, pre-scaled by sqrt(0.5)).
  2. |.|^2 on the scalar engine (all 128 partitions), bf16, stored s-minor.
  3. Kaiser pooling (uniform beta) on the PE: transpose [chan,time] ->
     [time,chan] chunks, then banded-kaiser matmuls accumulate
     pooledT[tp, chan] in persistent PSUM banks.
  4. PCEN scan as a decay-matrix matmul ema = pooled @ L, then the
     elementwise pow chain on ACT/DVE.
"""

import math
import os

import ml_dtypes
import numpy as np

SR = 16000
NF = 64
GK = 401
PK = 401
PSTRIDE = 160
PCEN_S = 0.025
FMIN = 30.0
FMAX = SR / 2.0 * 0.5
B, T = 8, 160000
TP = 1000
U = 1250  # T / 128
X2C = 1254  # x2 columns: u+d+2 for u<1250, d in [-2,2]
SEG_BOUNDS = [(0, 313), (313, 626), (626, 939), (939, 1250)]
N_CORES = 8

BF16 = ml_dtypes.bfloat16

# exposed for test.py
LAST_RESULT = None
LAST_NC = None
LAST_IN_MAPS = None


# ----------------------------------------------------------------- host math

def _softplus(x):
    return np.logaddexp(0.0, x)


def _host_filters(p_center, p_bw):
    """Wcat [128, 401] f32: rows 0-63 real, 64-127 imag, scaled by sqrt(0.5)."""
    half = (GK - 1) // 2
    t = np.arange(-half, half + 1, dtype=np.float64) / SR
    fc = np.clip(np.exp(p_center.astype(np.float64)), FMIN, FMAX - 10.0)
    bw_pos = _softplus(p_bw.astype(np.float64)) * 1000.0
    max_bw = 2.0 * np.minimum(fc - FMIN, FMAX - fc)
    bw = np.minimum(bw_pos, np.maximum(max_bw, 50.0))
    f_low = np.maximum(fc - 0.5 * bw, FMIN)
    f_high = np.minimum(fc + 0.5 * bw, FMAX)
    sigma = 0.5 / np.maximum(f_high - f_low, 20.0)
    env = np.exp(-0.5 * (t[None, :] / sigma[:, None]) ** 2)
    phase = 2.0 * np.pi * fc[:, None] * t[None, :]
    real_k = env * np.cos(phase)
    imag_k = env * np.sin(phase)
    W = np.concatenate([real_k, imag_k], axis=0) * np.sqrt(0.5)
    return W.astype(np.float32)


def _host_kaiser(beta):
    b = np.clip(beta.astype(np.float64), 1.0, 20.0)
    n = np.arange(PK, dtype=np.float64)
    arg = b[:, None] * np.sqrt(1.0 - (2.0 * n[None, :] / (PK - 1.0) - 1.0) ** 2)
    kais = np.i0(arg) / (np.i0(b)[:, None] + 1e-8)
    return kais.astype(np.float32)


def _valid_d(s):
    lo = int(math.ceil((s - 327) / 128))
    hi = (s + 200) // 128
    return list(range(lo, hi + 1))


def _build_weight_array(W):
    """W_all [128, ntiles*128] bf16, tiles ordered (s asc, d asc); returns
    (W_all, offsets) with offsets[s] = first tile index of s."""
    tiles = []
    offsets = []
    for s in range(128):
        offsets.append(len(tiles))
        for d in _valid_d(s):
            tile = np.zeros((128, 128), np.float32)
            q = np.arange(128)
            k = 128 * d + q + 200 - s
            msk = (k >= 0) & (k < GK)
            tile[msk, :] = W[:, k[msk]].T
            tiles.append(tile)
    W_all = np.concatenate(tiles, axis=1).astype(BF16)
    return W_all, offsets


def _build_kt_array(kr):
    """KT [128, 163*128] bf16; tile index o+2 for offset o in [-2, 160]:
    KT_o[q, m] = kr[128*o + q - 160*m + 200] (0 outside [0, 401))."""
    tiles = []
    for o in range(-2, 161):
        tile = np.zeros((128, 128), np.float32)
        for m in range(128):
            base = 128 * o - 160 * m + 200
            ks = np.arange(128) + base
            msk = (ks >= 0) & (ks < PK)
            tile[msk, m] = kr[ks[msk]]
        tiles.append(tile)
    return np.concatenate(tiles, axis=1).astype(BF16)


def _build_L():
    k_idx = np.arange(1024)
    tp_idx = np.arange(TP)
    Lm = np.where(
        (k_idx[:, None] <= tp_idx[None, :]) & (k_idx[:, None] < TP),
        PCEN_S * (1.0 - PCEN_S) ** np.clip(tp_idx[None, :] - k_idx[:, None], 0, None),
        0.0,
    )
    return Lm.astype(np.float32)


def _pool_blocks(c):
    """pooled blocks touched by time-chunk c."""
    b_lo = max(0, int(math.ceil((c - 160) / 160)))
    b_hi = min(7, (c + 2) // 160)
    return list(range(b_lo, b_hi + 1))


# ------------------------------------------------------------- device kernel

def _build_program():
    import concourse.bacc as bacc
    import concourse.bass as bass
    import concourse.mybir as mybir
    import concourse.tile as tile
    from concourse._compat import axon_active

    f32 = mybir.dt.float32
    bf16 = mybir.dt.bfloat16
    AF = mybir.ActivationFunctionType
    ALU = mybir.AluOpType

    n_wtiles = sum(len(_valid_d(s)) for s in range(128))
    woff = []
    acc = 0
    for s in range(128):
        woff.append(acc)
        acc += len(_valid_d(s))

    nc = bacc.Bacc(
        "TRN2",
        target_bir_lowering=False,
        debug=not axon_active(),
        num_devices=N_CORES,
    )

    x2_d = nc.dram_tensor("x2", [128, X2C], bf16, kind="ExternalInput").ap()
    w_d = nc.dram_tensor("W", [128, n_wtiles * 128], bf16, kind="ExternalInput").ap()
    kt_d = nc.dram_tensor("KT", [128, 163 * 128], bf16, kind="ExternalInput").ap()
    idb_d = nc.dram_tensor("IDB", [128, 128], bf16, kind="ExternalInput").ap()
    idf_d = nc.dram_tensor("IDF", [128, 128], f32, kind="ExternalInput").ap()
    par_d = nc.dram_tensor("PAR", [64, 5], f32, kind="ExternalInput").ap()
    l_d = nc.dram_tensor("L", [1024, TP], f32, kind="ExternalInput").ap()
    y_d = nc.dram_tensor("Y", [64, TP], f32, kind="ExternalOutput").ap()

    # first/last pooling contribution per psum bank, for start/stop flags
    bank_first = {}
    bank_last = {}
    for c in range(U):
        for blk in _pool_blocks(c):
            bank = blk // 4
            if bank not in bank_first:
                bank_first[bank] = (c, blk)
            bank_last[bank] = (c, blk)

    with tile.TileContext(nc) as tc:
        with (
            tc.tile_pool(name="const", bufs=1) as const_pool,
            tc.tile_pool(name="w", bufs=3) as wpool,
            tc.tile_pool(name="sq", bufs=1) as sq_pool,
            tc.tile_pool(name="sct", bufs=6) as sct_pool,
            tc.tile_pool(name="lp", bufs=2) as l_pool,
            tc.tile_pool(name="misc", bufs=1) as misc_pool,
            tc.tile_pool(name="psA", bufs=3, space="PSUM") as psA,
            tc.tile_pool(name="psB", bufs=2, space="PSUM") as psB,
            tc.tile_pool(name="psC", bufs=1, space="PSUM") as psC,
        ):
            x2_sb = const_pool.tile([128, X2C], bf16, tag="x2")
            nc.sync.dma_start(x2_sb[:], x2_d[:])
            kt_sb = const_pool.tile([128, 163 * 128], bf16, tag="kt")
            idb_sb = const_pool.tile([128, 128], bf16, tag="idb")
            idf_sb = const_pool.tile([128, 128], f32, tag="idf")
            par_sb = const_pool.tile([64, 5], f32, tag="par")

            pooled_ps = [
                psC.tile([128, 512], f32, tag=f"pool{i}", name=f"pool{i}") for i in range(2)
            ]

            for (u0, u1) in SEG_BOUNDS:
                useg = u1 - u0
                sq_seg = sq_pool.tile([128, 313 * 128], bf16, tag="sq", name="sq")
                sq_view = sq_seg[:].rearrange("p (u s) -> p u s", s=128)

                GS = 8
                for g in range(0, 128, GS):
                    g_lo = woff[g]
                    g_hi = woff[g + GS] if g + GS < 128 else n_wtiles
                    gw = g_hi - g_lo
                    wt = wpool.tile([128, 40 * 128], bf16, tag="w", name="wt")
                    nc.sync.dma_start(
                        wt[:, 0 : gw * 128],
                        w_d[:, g_lo * 128 : g_hi * 128],
                    )
                    for s in range(g, g + GS):
                        ds = _valid_d(s)
                        nt = len(ds)
                        toff = woff[s] - g_lo
                        cps = psA.tile([128, useg], f32, tag="conv", name="cps")
                        for di, d in enumerate(ds):
                            nc.tensor.matmul(
                                cps[:],
                                lhsT=wt[:, (toff + di) * 128 : (toff + di + 1) * 128],
                                rhs=x2_sb[:, u0 + d + 2 : u0 + d + 2 + useg],
                                start=(di == 0),
                                stop=(di == nt - 1),
                            )
                        nc.scalar.activation(
                            sq_view[:, 0:useg, s : s + 1], cps[:], AF.Square
                        )

                if u0 == 0:
                    # deferred const loads: queued after segment-0 conv weights
                    # so the first weight group isn't stuck behind 5.5 MB
                    nc.sync.dma_start(kt_sb[:], kt_d[:])
                    nc.sync.dma_start(idb_sb[:], idb_d[:])
                    nc.sync.dma_start(idf_sb[:], idf_d[:])
                    nc.sync.dma_start(par_sb[:], par_d[:])
                for cbase in range(u0, u1, 4):
                    n4 = min(4, u1 - cbase)
                    tp_ps = psB.tile([128, 512], bf16, tag="tp", name="tpps")
                    for j in range(n4):
                        cc = cbase - u0 + j
                        nc.tensor.transpose(
                            tp_ps[:, j * 128 : (j + 1) * 128],
                            sq_seg[:, cc * 128 : (cc + 1) * 128],
                            idb_sb[:],
                        )
                    sct = sct_pool.tile([128, 512], bf16, tag="sct", name="sct")
                    nc.vector.tensor_copy(
                        sct[:, 0 : n4 * 128], tp_ps[:, 0 : n4 * 128]
                    )
                    for j in range(n4):
                        c = cbase + j
                        for blk in _pool_blocks(c):
                            o = c - 160 * blk
                            bank = blk // 4
                            nc.tensor.matmul(
                                pooled_ps[bank][
                                    :, (blk % 4) * 128 : (blk % 4 + 1) * 128
                                ],
                                lhsT=kt_sb[:, (o + 2) * 128 : (o + 3) * 128],
                                rhs=sct[:, j * 128 : (j + 1) * 128],
                                start=(bank_first[bank] == (c, blk)),
                                stop=(bank_last[bank] == (c, blk)),
                                skip_group_check=True,
                            )

            # ---- PCEN tail ----
            pc = []
            for i in range(2):
                t = misc_pool.tile([128, 512], f32, tag=f"pc{i}", name=f"pc{i}")
                nc.vector.tensor_copy(t[:], pooled_ps[i][:])
                pc.append(t)
            poolsumT = misc_pool.tile([128, 512], f32, tag="pst")
            for blk in range(8):
                src = pc[blk // 4]
                col = (blk % 4) * 128
                nc.vector.tensor_add(
                    poolsumT[:, blk * 64 : (blk + 1) * 64],
                    src[:, col : col + 64],
                    src[:, col + 64 : col + 128],
                )

            ema_ps = [psA.tile([64, 500], f32, tag="conv", name=f"ema{_i}") for _i in range(2)]
            for blk in range(8):
                lt = l_pool.tile([128, TP], f32, tag="L", name="lt")
                nc.sync.dma_start(lt[:], l_d[blk * 128 : (blk + 1) * 128, :])
                for half in range(2):
                    nc.tensor.matmul(
                        ema_ps[half][:],
                        lhsT=poolsumT[:, blk * 64 : (blk + 1) * 64],
                        rhs=lt[:, half * 500 : (half + 1) * 500],
                        start=(blk == 0),
                        stop=(blk == 7),
                    )

            pnm_ps = [psB.tile([64, 512], f32, tag="tp", name=f"pnm{_i}") for _i in range(2)]
            for blk in range(8):
                nc.tensor.transpose(
                    pnm_ps[blk // 4][:, (blk % 4) * 128 : (blk % 4 + 1) * 128],
                    poolsumT[:, blk * 64 : (blk + 1) * 64],
                    idf_sb[:],
                )

            t0 = misc_pool.tile([64, TP], f32, tag="t0")
            for half in range(2):
                nc.scalar.activation(
                    t0[:, half * 500 : (half + 1) * 500],
                    ema_ps[half][:],
                    AF.Identity,
                    bias=par_sb[:, 4:5],
                )
            rec = misc_pool.tile([64, TP], f32, tag="rec")
            nc.vector.reciprocal(rec[:], t0[:])
            pnm = misc_pool.tile([64, TP], f32, tag="pnm")
            nc.scalar.copy(pnm[:, 0:512], pnm_ps[0][:])
            nc.scalar.copy(pnm[:, 512:TP], pnm_ps[1][:, 0:488])
            t2 = misc_pool.tile([64, TP], f32, tag="t2")
            nc.vector.tensor_mul(t2[:], pnm[:], rec[:])
            t3 = misc_pool.tile([64, TP], f32, tag="t3")
            nc.scalar.activation(t3[:], t2[:], AF.Ln, bias=par_sb[:, 0:1], scale=1.0)
            t4 = misc_pool.tile([64, TP], f32, tag="t4")
            nc.scalar.activation(t4[:], t3[:], AF.Exp, bias=0.0, scale=par_sb[:, 1:2])
            y_sb = misc_pool.tile([64, TP], f32, tag="y")
            nc.vector.tensor_scalar(
                y_sb[:], t4[:], par_sb[:, 2:3], par_sb[:, 3:4], ALU.mult, ALU.subtract
            )
            nc.sync.dma_start(y_d[:], y_sb[:])

    nc.compile()
    return nc


def _numpy_fallback(x, W, kais, pcen_g, pcen_o, pcen_e):
    """Correct-but-slow host path for non-uniform beta (never hit with the
    harness inputs, which use a uniform beta)."""
    out = np.zeros((B, NF, TP), np.float32)
    Wr, Wi = W[:NF] / np.sqrt(0.5), W[NF:] / np.sqrt(0.5)
    for b in range(B):
        xp = np.zeros(T + 2 * 200, np.float32)
        xp[200 : 200 + T] = x[b, 0]
        win = np.lib.stride_tricks.sliding_window_view(xp, GK)  # [T, GK]
        real = win @ Wr.T
        imag = win @ Wi.T
        scal = 0.5 * (real ** 2 + imag ** 2)  # [T, NF]
        sp = np.zeros((T + 2 * 200, NF), np.float32)
        sp[200 : 200 + T] = scal
        pooled = np.zeros((TP, NF), np.float32)
        for tp in range(TP):
            seg = sp[tp * PSTRIDE : tp * PSTRIDE + PK]
            pooled[tp] = np.einsum("kn,nk->n", seg, kais)
        g = np.clip(pcen_g, 0.5, 0.999)
        o = np.clip(pcen_o, 0.0, 10.0)
        e = np.clip(pcen_e, 0.1, 1.0)
        ema = np.zeros(NF, np.float32)
        for tp in range(TP):
            ema = (1.0 - PCEN_S) * ema + PCEN_S * pooled[tp]
            out[b, :, tp] = ((pooled[tp] / (ema + 1e-6) + o) ** e - o ** e) * g
    return out


def kernel(x, p_center, p_bw, beta, pcen_g, pcen_o, pcen_e):
    global LAST_RESULT
    x = np.asarray(x, np.float32)
    p_center = np.asarray(p_center, np.float32)
    p_bw = np.asarray(p_bw, np.float32)
    beta = np.asarray(beta, np.float32)
    pcen_g = np.asarray(pcen_g, np.float32)
    pcen_o = np.asarray(pcen_o, np.float32)
    pcen_e = np.asarray(pcen_e, np.float32)

    W = _host_filters(p_center, p_bw)
    kais = _host_kaiser(beta)
    if not np.all(kais == kais[0:1]):
        return _numpy_fallback(x, W, kais, pcen_g, pcen_o, pcen_e)

    W_all, _ = _build_weight_array(W)
    KT = _build_kt_array(kais[0])
    Lm = _build_L()
    g = np.clip(pcen_g, 0.5, 0.999)
    o = np.clip(pcen_o, 0.0, 10.0)
    e = np.clip(pcen_e, 0.1, 1.0)
    par = np.stack(
        [o, e, g, g * o ** e, np.full(NF, 1e-6, np.float32)], axis=1
    ).astype(np.float32)
    idb = np.eye(128, dtype=np.float32).astype(BF16)
    idf = np.eye(128, dtype=np.float32)

    x2s = []
    for b in range(B):
        xpad = np.zeros(128 * X2C, np.float32)
        xpad[256 : 256 + T] = x[b, 0]
        x2s.append(np.ascontiguousarray(xpad.reshape(X2C, 128).T).astype(BF16))

    nc = _build_program()

    shared = {"W": W_all, "KT": KT, "IDB": idb, "IDF": idf, "PAR": par, "L": Lm}
    in_maps = [dict(shared, x2=x2s[b]) for b in range(B)]
    global LAST_NC, LAST_IN_MAPS
    LAST_NC = nc
    LAST_IN_MAPS = in_maps

    from concourse.bass_utils import run_bass_kernel_spmd

    trace = bool(int(os.environ.get("DFBL_TRACE", "0")))
    res = run_bass_kernel_spmd(
        nc, in_maps, list(range(N_CORES)), trace=trace
    )
    LAST_RESULT = res
    out = np.stack([res.results[b]["Y"] for b in range(B)], axis=0)
    return out.astype(np.float32)



# revision 14
# speedup vs baseline: 1.0890x; 1.0890x over previous
"""Trainium2 Bass kernel for the DFBL (Gabor filterbank + Kaiser pooling + PCEN) model.

Contract: kernel(**inputs) takes the FULL unsharded inputs
(x [8,1,160000], six [64] param vectors) and returns the FULL output
[8, 64, 1000] float32. Internally shards batch across 8 NeuronCores.

Algorithm (per core, one batch element):
  1. Gabor conv as matmuls via the residue decomposition t = 128u + s:
     out[n, 128u+s] = sum_d Wsd[q,n].T @ x2[q, u+d], where
     x2[q, c] = xpad[128c + q] is a time-minor layout of x loaded once
     (no im2col DMA blowup), and Wsd are host-built 128x128 bf16 weight
     tiles (real|imag channel pairs, pre-scaled by sqrt(0.5)).
  2. |.|^2 on the scalar engine (all 128 partitions), bf16, stored s-minor.
  3. Kaiser pooling (uniform beta) on the PE: transpose [chan,time] ->
     [time,chan] chunks, then banded-kaiser matmuls accumulate
     pooledT[tp, chan] in persistent PSUM banks.
  4. PCEN scan as a decay-matrix matmul ema = pooled @ L, then the
     elementwise pow chain on ACT/DVE.
"""

import math
import os

import ml_dtypes
import numpy as np

SR = 16000
NF = 64
GK = 401
PK = 401
PSTRIDE = 160
PCEN_S = 0.025
FMIN = 30.0
FMAX = SR / 2.0 * 0.5
B, T = 8, 160000
TP = 1000
U = 1250  # T / 128
X2C = 1254  # x2 columns: u+d+2 for u<1250, d in [-2,2]
SEG_BOUNDS = [(0, 313), (313, 626), (626, 939), (939, 1250)]
N_CORES = 8

BF16 = ml_dtypes.bfloat16

# exposed for test.py
LAST_RESULT = None
LAST_NC = None
LAST_IN_MAPS = None


# ----------------------------------------------------------------- host math

def _softplus(x):
    return np.logaddexp(0.0, x)


def _host_filters(p_center, p_bw):
    """Wcat [128, 401] f32: rows 0-63 real, 64-127 imag, scaled by sqrt(0.5)."""
    half = (GK - 1) // 2
    t = np.arange(-half, half + 1, dtype=np.float64) / SR
    fc = np.clip(np.exp(p_center.astype(np.float64)), FMIN, FMAX - 10.0)
    bw_pos = _softplus(p_bw.astype(np.float64)) * 1000.0
    max_bw = 2.0 * np.minimum(fc - FMIN, FMAX - fc)
    bw = np.minimum(bw_pos, np.maximum(max_bw, 50.0))
    f_low = np.maximum(fc - 0.5 * bw, FMIN)
    f_high = np.minimum(fc + 0.5 * bw, FMAX)
    sigma = 0.5 / np.maximum(f_high - f_low, 20.0)
    env = np.exp(-0.5 * (t[None, :] / sigma[:, None]) ** 2)
    phase = 2.0 * np.pi * fc[:, None] * t[None, :]
    real_k = env * np.cos(phase)
    imag_k = env * np.sin(phase)
    W = np.concatenate([real_k, imag_k], axis=0) * np.sqrt(0.5)
    return W.astype(np.float32)


def _host_kaiser(beta):
    b = np.clip(beta.astype(np.float64), 1.0, 20.0)
    n = np.arange(PK, dtype=np.float64)
    arg = b[:, None] * np.sqrt(1.0 - (2.0 * n[None, :] / (PK - 1.0) - 1.0) ** 2)
    kais = np.i0(arg) / (np.i0(b)[:, None] + 1e-8)
    return kais.astype(np.float32)


def _valid_d(s):
    lo = int(math.ceil((s - 327) / 128))
    hi = (s + 200) // 128
    return list(range(lo, hi + 1))


def _build_weight_array(W):
    """W_all [128, ntiles*128] bf16, tiles ordered (s asc, d asc); returns
    (W_all, offsets) with offsets[s] = first tile index of s."""
    tiles = []
    offsets = []
    for s in range(128):
        offsets.append(len(tiles))
        for d in _valid_d(s):
            tile = np.zeros((128, 128), np.float32)
            q = np.arange(128)
            k = 128 * d + q + 200 - s
            msk = (k >= 0) & (k < GK)
            tile[msk, :] = W[:, k[msk]].T
            tiles.append(tile)
    W_all = np.concatenate(tiles, axis=1).astype(BF16)
    return W_all, offsets


def _build_kt_array(kr):
    """KT [128, 163*128] bf16; tile index o+2 for offset o in [-2, 160]:
    KT_o[q, m] = kr[128*o + q - 160*m + 200] (0 outside [0, 401))."""
    tiles = []
    for o in range(-2, 161):
        tile = np.zeros((128, 128), np.float32)
        for m in range(128):
            base = 128 * o - 160 * m + 200
            ks = np.arange(128) + base
            msk = (ks >= 0) & (ks < PK)
            tile[msk, m] = kr[ks[msk]]
        tiles.append(tile)
    return np.concatenate(tiles, axis=1).astype(BF16)


def _build_L():
    k_idx = np.arange(1024)
    tp_idx = np.arange(TP)
    Lm = np.where(
        (k_idx[:, None] <= tp_idx[None, :]) & (k_idx[:, None] < TP),
        PCEN_S * (1.0 - PCEN_S) ** np.clip(tp_idx[None, :] - k_idx[:, None], 0, None),
        0.0,
    )
    return Lm.astype(np.float32)


def _pool_blocks(c):
    """pooled blocks touched by time-chunk c."""
    b_lo = max(0, int(math.ceil((c - 160) / 160)))
    b_hi = min(7, (c + 2) // 160)
    return list(range(b_lo, b_hi + 1))


# ------------------------------------------------------------- device kernel

def _build_program(reps=1, stages=('conv', 'pool', 'pcen')):
    """reps>1 wraps the whole body in a hardware For_i loop: used by test.py
    to measure per-iteration HW time as a slope, cancelling the large and
    noisy axon dispatch floor. The computed output is identical."""
    import contextlib

    import concourse.bacc as bacc
    import concourse.bass as bass
    import concourse.mybir as mybir
    import concourse.tile as tile
    from concourse._compat import axon_active

    f32 = mybir.dt.float32
    bf16 = mybir.dt.bfloat16
    AF = mybir.ActivationFunctionType
    ALU = mybir.AluOpType

    n_wtiles = sum(len(_valid_d(s)) for s in range(128))
    woff = []
    acc = 0
    for s in range(128):
        woff.append(acc)
        acc += len(_valid_d(s))

    nc = bacc.Bacc(
        "TRN2",
        target_bir_lowering=False,
        debug=not axon_active(),
        num_devices=N_CORES,
    )

    x2_d = nc.dram_tensor("x2", [128, X2C], bf16, kind="ExternalInput").ap()
    w_d = nc.dram_tensor("W", [128, n_wtiles * 128], bf16, kind="ExternalInput").ap()
    kt_d = nc.dram_tensor("KT", [128, 163 * 128], bf16, kind="ExternalInput").ap()
    idb_d = nc.dram_tensor("IDB", [128, 128], bf16, kind="ExternalInput").ap()
    idf_d = nc.dram_tensor("IDF", [128, 128], f32, kind="ExternalInput").ap()
    par_d = nc.dram_tensor("PAR", [64, 5], f32, kind="ExternalInput").ap()
    l_d = nc.dram_tensor("L", [1024, TP], f32, kind="ExternalInput").ap()
    y_d = nc.dram_tensor("Y", [64, TP], f32, kind="ExternalOutput").ap()

    # first/last pooling contribution per psum bank, for start/stop flags
    bank_first = {}
    bank_last = {}
    for c in range(U):
        for blk in _pool_blocks(c):
            bank = blk // 4
            if bank not in bank_first:
                bank_first[bank] = (c, blk)
            bank_last[bank] = (c, blk)

    with tile.TileContext(nc) as tc:
        with (
            tc.tile_pool(name="const", bufs=1) as const_pool,
            tc.tile_pool(name="w", bufs=3) as wpool,
            tc.tile_pool(name="sq", bufs=1) as sq_pool,
            tc.tile_pool(name="sct", bufs=6) as sct_pool,
            tc.tile_pool(name="lp", bufs=2) as l_pool,
            tc.tile_pool(name="misc", bufs=1) as misc_pool,
            tc.tile_pool(name="psA", bufs=3, space="PSUM") as psA,
            tc.tile_pool(name="psB", bufs=2, space="PSUM") as psB,
            tc.tile_pool(name="psC", bufs=1, space="PSUM") as psC,
        ):
            x2_sb = const_pool.tile([128, X2C], bf16, tag="x2")
            nc.sync.dma_start(x2_sb[:], x2_d[:])
            kt_sb = const_pool.tile([128, 163 * 128], bf16, tag="kt")
            idb_sb = const_pool.tile([128, 128], bf16, tag="idb")
            idf_sb = const_pool.tile([128, 128], f32, tag="idf")
            par_sb = const_pool.tile([64, 5], f32, tag="par")

            pooled_ps = [
                psC.tile([128, 512], f32, tag=f"pool{i}", name=f"pool{i}") for i in range(2)
            ]

            loop_cm = tc.For_i(0, reps, 1) if reps > 1 else contextlib.nullcontext()
            with loop_cm:
                _emit_body(
                    nc, tc, mybir, f32, bf16, AF, ALU, woff, n_wtiles,
                    x2_sb, kt_sb, idb_sb, idf_sb, par_sb, pooled_ps,
                    wpool, sq_pool, sct_pool, l_pool, misc_pool,
                    psA, psB, psC, w_d, l_d, y_d, kt_d, idb_d, idf_d,
                    par_d, bank_first, bank_last, stages,
                )

    nc.compile()
    return nc


def _emit_body(
    nc, tc, mybir, f32, bf16, AF, ALU, woff, n_wtiles,
    x2_sb, kt_sb, idb_sb, idf_sb, par_sb, pooled_ps,
    wpool, sq_pool, sct_pool, l_pool, misc_pool,
    psA, psB, psC, w_d, l_d, y_d, kt_d, idb_d, idf_d,
    par_d, bank_first, bank_last, stages=('conv', 'pool', 'pcen'),
):
    if True:
        if True:
            for (u0, u1) in SEG_BOUNDS:
                useg = u1 - u0
                sq_seg = sq_pool.tile([128, 313 * 128], bf16, tag="sq", name="sq")
                sq_view = sq_seg[:].rearrange("p (u s) -> p u s", s=128)

                GS = 8
                for g in range(0, 128, GS):
                    g_lo = woff[g]
                    g_hi = woff[g + GS] if g + GS < 128 else n_wtiles
                    gw = g_hi - g_lo
                    wt = wpool.tile([128, 40 * 128], bf16, tag="w", name="wt")
                    nc.sync.dma_start(
                        wt[:, 0 : gw * 128],
                        w_d[:, g_lo * 128 : g_hi * 128],
                    )
                    for s in range(g, g + GS):
                        ds = _valid_d(s)
                        nt = len(ds)
                        toff = woff[s] - g_lo
                        cps = psA.tile([128, useg], f32, tag="conv", name="cps")
                        for di, d in enumerate(ds):
                            nc.tensor.matmul(
                                cps[:],
                                lhsT=wt[:, (toff + di) * 128 : (toff + di + 1) * 128],
                                rhs=x2_sb[:, u0 + d + 2 : u0 + d + 2 + useg],
                                start=(di == 0),
                                stop=(di == nt - 1),
                            )
                        nc.scalar.activation(
                            sq_view[:, 0:useg, s : s + 1], cps[:], AF.Square
                        )

                if u0 == 0:
                    # deferred const loads: queued after segment-0 conv weights
                    # so the first weight group isn't stuck behind 5.5 MB
                    nc.sync.dma_start(kt_sb[:], kt_d[:])
                    nc.sync.dma_start(idb_sb[:], idb_d[:])
                    nc.sync.dma_start(idf_sb[:], idf_d[:])
                    nc.sync.dma_start(par_sb[:], par_d[:])
                for cbase in (range(u0, u1, 4) if "pool" in stages else ()):
                    n4 = min(4, u1 - cbase)
                    tp_ps = psB.tile([128, 512], bf16, tag="tp", name="tpps")
                    for j in range(n4):
                        cc = cbase - u0 + j
                        nc.tensor.transpose(
                            tp_ps[:, j * 128 : (j + 1) * 128],
                            sq_seg[:, cc * 128 : (cc + 1) * 128],
                            idb_sb[:],
                        )
                    sct = sct_pool.tile([128, 512], bf16, tag="sct", name="sct")
                    nc.vector.tensor_copy(
                        sct[:, 0 : n4 * 128], tp_ps[:, 0 : n4 * 128]
                    )
                    for j in range(n4):
                        c = cbase + j
                        for blk in _pool_blocks(c):
                            o = c - 160 * blk
                            bank = blk // 4
                            nc.tensor.matmul(
                                pooled_ps[bank][
                                    :, (blk % 4) * 128 : (blk % 4 + 1) * 128
                                ],
                                lhsT=kt_sb[:, (o + 2) * 128 : (o + 3) * 128],
                                rhs=sct[:, j * 128 : (j + 1) * 128],
                                start=(bank_first[bank] == (c, blk)),
                                stop=(bank_last[bank] == (c, blk)),
                                skip_group_check=True,
                            )

            # ---- PCEN tail ----
            if "pcen" not in stages:
                y_sb = misc_pool.tile([64, TP], f32, tag="y")
                if "pool" in stages:
                    nc.vector.tensor_copy(y_sb[:, 0:500], pooled_ps[0][0:64, 0:500])
                    nc.vector.tensor_copy(y_sb[:, 500:TP], pooled_ps[1][0:64, 0:500])
                else:
                    nc.vector.tensor_copy(y_sb[:], x2_sb[0:64, 0:TP])
                nc.sync.dma_start(y_d[:], y_sb[:])
                return
            pc = []
            for i in range(2):
                t = misc_pool.tile([128, 512], f32, tag=f"pc{i}", name=f"pc{i}")
                nc.vector.tensor_copy(t[:], pooled_ps[i][:])
                pc.append(t)
            poolsumT = misc_pool.tile([128, 512], f32, tag="pst")
            for blk in range(8):
                src = pc[blk // 4]
                col = (blk % 4) * 128
                nc.vector.tensor_add(
                    poolsumT[:, blk * 64 : (blk + 1) * 64],
                    src[:, col : col + 64],
                    src[:, col + 64 : col + 128],
                )

            ema_ps = [psA.tile([64, 500], f32, tag="conv", name=f"ema{_i}") for _i in range(2)]
            for blk in range(8):
                lt = l_pool.tile([128, TP], f32, tag="L", name="lt")
                nc.sync.dma_start(lt[:], l_d[blk * 128 : (blk + 1) * 128, :])
                for half in range(2):
                    nc.tensor.matmul(
                        ema_ps[half][:],
                        lhsT=poolsumT[:, blk * 64 : (blk + 1) * 64],
                        rhs=lt[:, half * 500 : (half + 1) * 500],
                        start=(blk == 0),
                        stop=(blk == 7),
                    )

            pnm_ps = [psB.tile([64, 512], f32, tag="tp", name=f"pnm{_i}") for _i in range(2)]
            for blk in range(8):
                nc.tensor.transpose(
                    pnm_ps[blk // 4][:, (blk % 4) * 128 : (blk % 4 + 1) * 128],
                    poolsumT[:, blk * 64 : (blk + 1) * 64],
                    idf_sb[:],
                )

            t0 = misc_pool.tile([64, TP], f32, tag="t0")
            for half in range(2):
                nc.scalar.activation(
                    t0[:, half * 500 : (half + 1) * 500],
                    ema_ps[half][:],
                    AF.Identity,
                    bias=par_sb[:, 4:5],
                )
            rec = misc_pool.tile([64, TP], f32, tag="rec")
            nc.vector.reciprocal(rec[:], t0[:])
            pnm = misc_pool.tile([64, TP], f32, tag="pnm")
            nc.scalar.copy(pnm[:, 0:512], pnm_ps[0][:])
            nc.scalar.copy(pnm[:, 512:TP], pnm_ps[1][:, 0:488])
            t2 = misc_pool.tile([64, TP], f32, tag="t2")
            nc.vector.tensor_mul(t2[:], pnm[:], rec[:])
            t3 = misc_pool.tile([64, TP], f32, tag="t3")
            nc.scalar.activation(t3[:], t2[:], AF.Ln, bias=par_sb[:, 0:1], scale=1.0)
            t4 = misc_pool.tile([64, TP], f32, tag="t4")
            nc.scalar.activation(t4[:], t3[:], AF.Exp, bias=0.0, scale=par_sb[:, 1:2])
            y_sb = misc_pool.tile([64, TP], f32, tag="y")
            nc.vector.tensor_scalar(
                y_sb[:], t4[:], par_sb[:, 2:3], par_sb[:, 3:4], ALU.mult, ALU.subtract
            )
            nc.sync.dma_start(y_d[:], y_sb[:])


# ------------------------------------------------------- v2 device kernel
# fp8(e4m3) DoubleRow conv: per s, consecutive-d weight tiles are paired into
# [128, 2, 128] stationary planes (K=256 per matmul, 0.5 cycles/row); the
# moving operand is x2 with its two planes the d and d+1 column shifts
# (overlapping stride-1 AP). x is split hi/lo into two fp8 tensors
# (x = hi + lo), each streamed through the same pair, restoring ~bf16 input
# precision. 3 segments of ~417 halve the ldweights count vs 4x313.
# The chunk transpose is fused with the real^2+imag^2 pair-add as a plain
# matmul against a 2-hot pairing matrix (64-col output), which also halves
# the pooling matmul streams.

SEG_BOUNDS2 = [(0, 417), (417, 834), (834, 1250)]
X2C2 = 1256


def _pairs_of(s):
    ds = _valid_d(s)
    ps = []
    for i in range(0, len(ds) - 1, 2):
        ps.append((ds[i], ds[i + 1]))
    if len(ds) % 2:
        ps.append((ds[-1], None))
    return ps


def _build_weight_pairs(W):
    """W_pairs [128, npairs*256] fp8e4; pair tiles (d0, d1) adjacent; odd
    counts padded with a zero tile. Returns (array, poff) with poff[s] the
    first pair index of s."""
    E4 = ml_dtypes.float8_e4m3
    cols = []
    poff = []
    np_ = 0
    for s in range(128):
        poff.append(np_)
        for (d0, d1) in _pairs_of(s):
            for d in (d0, d1):
                tile = np.zeros((128, 128), np.float32)
                if d is not None:
                    q = np.arange(128)
                    k = 128 * d + q + 200 - s
                    msk = (k >= 0) & (k < GK)
                    tile[msk, :] = W[:, k[msk]].T
                cols.append(tile)
            np_ += 1
    return np.concatenate(cols, axis=1).astype(E4), poff


XP = 1280  # plane pitch for the DoubleRow moving operand (16-aligned)


def _build_x2_hilo(xb):
    """x2 hi/lo fp8 split, each stored twice: plane 0 = x2, plane 1 = x2
    shifted left one column (the d+1 tap block), at pitch XP so the
    DoubleRow rhs is a clean non-overlapping 3D AP."""
    E4 = ml_dtypes.float8_e4m3
    xpad = np.zeros(128 * X2C2, np.float32)
    xpad[256 : 256 + T] = xb
    x2 = np.ascontiguousarray(xpad.reshape(X2C2, 128).T)
    x2h = x2.astype(E4)
    x2l = (x2 - x2h.astype(np.float32)).astype(E4)
    out = []
    for a in (x2h, x2l):
        cat = np.zeros((128, 2 * XP), E4)
        cat[:, 0:X2C2] = a
        cat[:, XP : XP + X2C2 - 1] = a[:, 1:]
        out.append(cat)
    return out[0], out[1]


def _build_program_v2(reps=1):
    import contextlib

    import concourse.bacc as bacc
    import concourse.mybir as mybir
    import concourse.tile as tile
    from concourse._compat import axon_active
    from concourse.ap import AP as APClass

    f32 = mybir.dt.float32
    bf16 = mybir.dt.bfloat16
    fp8 = mybir.dt.float8e4
    AF = mybir.ActivationFunctionType
    ALU = mybir.AluOpType
    DR = mybir.MatmulPerfMode.DoubleRow

    pair_lists = [_pairs_of(s) for s in range(128)]
    poff = []
    acc = 0
    for s in range(128):
        poff.append(acc)
        acc += len(pair_lists[s])
    n_pairs = acc

    nc = bacc.Bacc(
        "TRN2",
        target_bir_lowering=False,
        debug=not axon_active(),
        num_devices=N_CORES,
    )

    x2h_d = nc.dram_tensor("x2h", [128, 2 * XP], fp8, kind="ExternalInput").ap()
    x2l_d = nc.dram_tensor("x2l", [128, 2 * XP], fp8, kind="ExternalInput").ap()
    w_d = nc.dram_tensor("W", [128, n_pairs * 256], fp8, kind="ExternalInput").ap()
    kt_d = nc.dram_tensor("KT", [128, 163 * 128], bf16, kind="ExternalInput").ap()
    pm_d = nc.dram_tensor("PM", [128, 64], bf16, kind="ExternalInput").ap()
    idf_d = nc.dram_tensor("IDF", [128, 128], f32, kind="ExternalInput").ap()
    par_d = nc.dram_tensor("PAR", [64, 5], f32, kind="ExternalInput").ap()
    l_d = nc.dram_tensor("L", [1024, TP], f32, kind="ExternalInput").ap()
    y_d = nc.dram_tensor("Y", [64, TP], f32, kind="ExternalOutput").ap()

    bank_first = {}
    bank_last = {}
    for c in range(U):
        for blk in _pool_blocks(c):
            bank = blk // 4
            if bank not in bank_first:
                bank_first[bank] = (c, blk)
            bank_last[bank] = (c, blk)

    def dr_rhs(x2sb, col0, useg):
        v3 = x2sb[:].rearrange("p (two c) -> p two c", two=2)
        return v3[:, :, col0 : col0 + useg]

    with tile.TileContext(nc) as tc:
        with (
            tc.tile_pool(name="const", bufs=1) as const_pool,
            tc.tile_pool(name="w", bufs=3) as wpool,
            tc.tile_pool(name="sq", bufs=1) as sq_pool,
            tc.tile_pool(name="sct", bufs=6) as sct_pool,
            tc.tile_pool(name="lp", bufs=1) as l_pool,
            tc.tile_pool(name="misc", bufs=1) as misc_pool,
            tc.tile_pool(name="psA", bufs=3, space="PSUM") as psA,
            tc.tile_pool(name="psB", bufs=2, space="PSUM") as psB,
            tc.tile_pool(name="psC", bufs=1, space="PSUM") as psC,
        ):
            x2h_sb = const_pool.tile([128, 2 * XP], fp8, tag="x2h")
            nc.sync.dma_start(x2h_sb[:], x2h_d[:])
            x2l_sb = const_pool.tile([128, 2 * XP], fp8, tag="x2l")
            nc.sync.dma_start(x2l_sb[:], x2l_d[:])
            kt_sb = const_pool.tile([128, 163 * 128], bf16, tag="kt")
            pm_sb = const_pool.tile([128, 64], bf16, tag="pm")
            idf_sb = const_pool.tile([128, 128], f32, tag="idf")
            par_sb = const_pool.tile([64, 5], f32, tag="par")

            pooled_ps = [
                psC.tile([128, 256], f32, tag=f"pool{i}", name=f"pool{i}")
                for i in range(2)
            ]

            loop_cm = tc.For_i(0, reps, 1) if reps > 1 else contextlib.nullcontext()
            with loop_cm:
                for si, (u0, u1) in enumerate(SEG_BOUNDS2):
                    useg = u1 - u0
                    sq_seg = sq_pool.tile([128, 417 * 128], bf16, tag="sq", name="sq")
                    sq_view = sq_seg[:].rearrange("p (u s) -> p u s", s=128)

                    GS = 8
                    for g in range(0, 128, GS):
                        g_lo = poff[g]
                        g_hi = poff[g + GS] if g + GS < 128 else n_pairs
                        gw = g_hi - g_lo
                        wt = wpool.tile([128, 24 * 256], fp8, tag="w", name="wt")
                        nc.sync.dma_start(
                            wt[:, 0 : gw * 256],
                            w_d[:, g_lo * 256 : g_hi * 256],
                        )
                        for s in range(g, g + GS):
                            prs = pair_lists[s]
                            npr = len(prs)
                            toff = poff[s] - g_lo
                            cps = psA.tile([128, useg], f32, tag="conv", name="cps")
                            for pi, (d0, _d1) in enumerate(prs):
                                wpair = wt[
                                    :, (toff + pi) * 256 : (toff + pi + 1) * 256
                                ].rearrange("p (two c) -> p two c", two=2)
                                col0 = u0 + d0 + 2
                                nc.tensor.matmul(
                                    cps[:],
                                    lhsT=wpair,
                                    rhs=dr_rhs(x2h_sb, col0, useg),
                                    start=(pi == 0),
                                    stop=False,
                                    perf_mode=DR,
                                )
                                nc.tensor.matmul(
                                    cps[:],
                                    lhsT=wpair,
                                    rhs=dr_rhs(x2l_sb, col0, useg),
                                    start=False,
                                    stop=(pi == npr - 1),
                                    perf_mode=DR,
                                )
                            nc.scalar.activation(
                                sq_view[:, 0:useg, s : s + 1], cps[:], AF.Square
                            )

                    if si == 0:
                        # deferred const loads behind the first weight group
                        nc.sync.dma_start(kt_sb[:], kt_d[:])
                        nc.sync.dma_start(pm_sb[:], pm_d[:])
                        nc.sync.dma_start(idf_sb[:], idf_d[:])
                        nc.sync.dma_start(par_sb[:], par_d[:])
                    for cbase in range(u0, u1, 4):
                        n4 = min(4, u1 - cbase)
                        tp_ps = psB.tile([128, 256], f32, tag="tp", name="tpps")
                        for j in range(n4):
                            cc = cbase - u0 + j
                            # fused transpose + (re^2 + im^2): out[t, j64] =
                            # sum_c sq[c, t] * PM[c, j64]
                            nc.tensor.matmul(
                                tp_ps[:, j * 64 : (j + 1) * 64],
                                lhsT=sq_seg[:, cc * 128 : (cc + 1) * 128],
                                rhs=pm_sb[:],
                                start=True,
                                stop=True,
                                skip_group_check=True,
                            )
                        sct = sct_pool.tile([128, 256], bf16, tag="sct", name="sct")
                        nc.vector.tensor_copy(
                            sct[:, 0 : n4 * 64], tp_ps[:, 0 : n4 * 64]
                        )
                        for j in range(n4):
                            c = cbase + j
                            for blk in _pool_blocks(c):
                                o = c - 160 * blk
                                bank = blk // 4
                                nc.tensor.matmul(
                                    pooled_ps[bank][
                                        :, (blk % 4) * 64 : (blk % 4 + 1) * 64
                                    ],
                                    lhsT=kt_sb[:, (o + 2) * 128 : (o + 3) * 128],
                                    rhs=sct[:, j * 64 : (j + 1) * 64],
                                    start=(bank_first[bank] == (c, blk)),
                                    stop=(bank_last[bank] == (c, blk)),
                                    skip_group_check=True,
                                )

                # ---- PCEN tail ----
                poolsumT = misc_pool.tile([128, 512], f32, tag="pst")
                for i in range(2):
                    nc.vector.tensor_copy(
                        poolsumT[:, i * 256 : (i + 1) * 256], pooled_ps[i][:]
                    )

                ema_ps = [
                    psA.tile([64, 500], f32, tag="conv", name=f"ema{_i}")
                    for _i in range(2)
                ]
                for blk in range(8):
                    lt = l_pool.tile([128, TP], f32, tag="L", name="lt")
                    nc.sync.dma_start(lt[:], l_d[blk * 128 : (blk + 1) * 128, :])
                    for half in range(2):
                        nc.tensor.matmul(
                            ema_ps[half][:],
                            lhsT=poolsumT[:, blk * 64 : (blk + 1) * 64],
                            rhs=lt[:, half * 500 : (half + 1) * 500],
                            start=(blk == 0),
                            stop=(blk == 7),
                        )

                pnm_ps = [
                    psB.tile([64, 512], f32, tag="tp", name=f"pnm{_i}")
                    for _i in range(2)
                ]
                for blk in range(8):
                    nc.tensor.transpose(
                        pnm_ps[blk // 4][:, (blk % 4) * 128 : (blk % 4 + 1) * 128],
                        poolsumT[:, blk * 64 : (blk + 1) * 64],
                        idf_sb[:],
                    )

                t0 = misc_pool.tile([64, TP], f32, tag="t0")
                for half in range(2):
                    nc.scalar.activation(
                        t0[:, half * 500 : (half + 1) * 500],
                        ema_ps[half][:],
                        AF.Identity,
                        bias=par_sb[:, 4:5],
                    )
                rec = misc_pool.tile([64, TP], f32, tag="rec")
                nc.vector.reciprocal(rec[:], t0[:])
                pnm = misc_pool.tile([64, TP], f32, tag="pnm")
                nc.scalar.copy(pnm[:, 0:512], pnm_ps[0][:])
                nc.scalar.copy(pnm[:, 512:TP], pnm_ps[1][:, 0:488])
                t2 = misc_pool.tile([64, TP], f32, tag="t2")
                nc.vector.tensor_mul(t2[:], pnm[:], rec[:])
                t3 = misc_pool.tile([64, TP], f32, tag="t3")
                nc.scalar.activation(t3[:], t2[:], AF.Ln, bias=par_sb[:, 0:1], scale=1.0)
                t4 = misc_pool.tile([64, TP], f32, tag="t4")
                nc.scalar.activation(t4[:], t3[:], AF.Exp, bias=0.0, scale=par_sb[:, 1:2])
                y_sb = misc_pool.tile([64, TP], f32, tag="y")
                nc.vector.tensor_scalar(
                    y_sb[:], t4[:], par_sb[:, 2:3], par_sb[:, 3:4], ALU.mult, ALU.subtract
                )
                nc.sync.dma_start(y_d[:], y_sb[:])

    nc.compile()
    return nc


def _v2_in_maps(x, W, kais, pcen_g, pcen_o, pcen_e):
    W_pairs, _ = _build_weight_pairs(W)
    KT = _build_kt_array(kais[0])
    Lm = _build_L()
    g = np.clip(pcen_g, 0.5, 0.999)
    o = np.clip(pcen_o, 0.0, 10.0)
    e = np.clip(pcen_e, 0.1, 1.0)
    par = np.stack(
        [o, e, g, g * o ** e, np.full(NF, 1e-6, np.float32)], axis=1
    ).astype(np.float32)
    pm = np.zeros((128, 64), np.float32)
    pm[np.arange(64), np.arange(64)] = 1.0
    pm[np.arange(64) + 64, np.arange(64)] = 1.0
    idf = np.eye(128, dtype=np.float32)
    shared = {
        "W": W_pairs, "KT": KT, "PM": pm.astype(BF16), "IDF": idf,
        "PAR": par, "L": Lm,
    }
    maps = []
    for b in range(B):
        x2h, x2l = _build_x2_hilo(x[b, 0])
        maps.append(dict(shared, x2h=x2h, x2l=x2l))
    return maps


def _numpy_fallback(x, W, kais, pcen_g, pcen_o, pcen_e):
    """Correct-but-slow host path for non-uniform beta (never hit with the
    harness inputs, which use a uniform beta)."""
    out = np.zeros((B, NF, TP), np.float32)
    Wr, Wi = W[:NF] / np.sqrt(0.5), W[NF:] / np.sqrt(0.5)
    for b in range(B):
        xp = np.zeros(T + 2 * 200, np.float32)
        xp[200 : 200 + T] = x[b, 0]
        win = np.lib.stride_tricks.sliding_window_view(xp, GK)  # [T, GK]
        real = win @ Wr.T
        imag = win @ Wi.T
        scal = 0.5 * (real ** 2 + imag ** 2)  # [T, NF]
        sp = np.zeros((T + 2 * 200, NF), np.float32)
        sp[200 : 200 + T] = scal
        pooled = np.zeros((TP, NF), np.float32)
        for tp in range(TP):
            seg = sp[tp * PSTRIDE : tp * PSTRIDE + PK]
            pooled[tp] = np.einsum("kn,nk->n", seg, kais)
        g = np.clip(pcen_g, 0.5, 0.999)
        o = np.clip(pcen_o, 0.0, 10.0)
        e = np.clip(pcen_e, 0.1, 1.0)
        ema = np.zeros(NF, np.float32)
        for tp in range(TP):
            ema = (1.0 - PCEN_S) * ema + PCEN_S * pooled[tp]
            out[b, :, tp] = ((pooled[tp] / (ema + 1e-6) + o) ** e - o ** e) * g
    return out


def kernel(x, p_center, p_bw, beta, pcen_g, pcen_o, pcen_e):
    global LAST_RESULT, LAST_NC, LAST_IN_MAPS
    x = np.asarray(x, np.float32)
    p_center = np.asarray(p_center, np.float32)
    p_bw = np.asarray(p_bw, np.float32)
    beta = np.asarray(beta, np.float32)
    pcen_g = np.asarray(pcen_g, np.float32)
    pcen_o = np.asarray(pcen_o, np.float32)
    pcen_e = np.asarray(pcen_e, np.float32)

    W = _host_filters(p_center, p_bw)
    kais = _host_kaiser(beta)
    if not np.all(kais == kais[0:1]):
        return _numpy_fallback(x, W, kais, pcen_g, pcen_o, pcen_e)

    if not bool(int(os.environ.get("DFBL_V1", "0"))):
        in_maps = _v2_in_maps(x, W, kais, pcen_g, pcen_o, pcen_e)
        nc = _build_program_v2()
        LAST_NC = nc
        LAST_IN_MAPS = in_maps

        from concourse.bass_utils import run_bass_kernel_spmd

        res = run_bass_kernel_spmd(nc, in_maps, list(range(N_CORES)))
        LAST_RESULT = res
        out = np.stack([res.results[b]["Y"] for b in range(B)], axis=0)
        return out.astype(np.float32)

    W_all, _ = _build_weight_array(W)
    KT = _build_kt_array(kais[0])
    Lm = _build_L()
    g = np.clip(pcen_g, 0.5, 0.999)
    o = np.clip(pcen_o, 0.0, 10.0)
    e = np.clip(pcen_e, 0.1, 1.0)
    par = np.stack(
        [o, e, g, g * o ** e, np.full(NF, 1e-6, np.float32)], axis=1
    ).astype(np.float32)
    idb = np.eye(128, dtype=np.float32).astype(BF16)
    idf = np.eye(128, dtype=np.float32)

    x2s = []
    for b in range(B):
        xpad = np.zeros(128 * X2C, np.float32)
        xpad[256 : 256 + T] = x[b, 0]
        x2s.append(np.ascontiguousarray(xpad.reshape(X2C, 128).T).astype(BF16))

    nc = _build_program()

    shared = {"W": W_all, "KT": KT, "IDB": idb, "IDF": idf, "PAR": par, "L": Lm}
    in_maps = [dict(shared, x2=x2s[b]) for b in range(B)]
    LAST_NC = nc
    LAST_IN_MAPS = in_maps

    from concourse.bass_utils import run_bass_kernel_spmd

    trace = bool(int(os.environ.get("DFBL_TRACE", "0")))
    res = run_bass_kernel_spmd(
        nc, in_maps, list(range(N_CORES)), trace=trace
    )
    LAST_RESULT = res
    out = np.stack([res.results[b]["Y"] for b in range(B)], axis=0)
    return out.astype(np.float32)

